# revision 12
# baseline (speedup 1.0000x reference)
"""AttentionSSA Trainium2 Bass kernel.

Computation (per batch b):
  qkv = x @ qkv_w + qkv_b ; split into per-head q,k,v
  S = (q @ k^T) * scale
  attn = softmax(w)[0] * softmax(S) + softmax(w)[1] * relu(S)^2
  out = (attn @ v) reassembled, @ proj_w + proj_b

Sharding: data-parallel over batch B=16 across 8 NeuronCores (2 batches/core).
Each core computes its slice fully independently (no collectives).

Per-core dataflow:
  P1: x_b [640,768] --PE transpose--> xT_b [768,640]
  P2: qT,kT = (qkv_w.T @ x.T) per feature tile (T-orientation, [feat, tok]);
      bias folded into the PSUM->SBUF eviction (ACT Identity with per-
      partition bias column). v in natural orientation [tok, feat];
      evicted twice: VR0 = [w0*v | 1] (per-head strided) and
      VR1 = (w1*SCALE^2)*v (contiguous).
  P3: per (b, h): ST[ktok, qtok] = k q^T;
      p0 = exp(SCALE*ST)            (ACT, fused scale)
      p1 = relu(ST)^2 = (ST max 0)*ST  (DVE fused scalar_tensor_tensor;
                                        one kt per head via ACT relu +
                                        gpsimd square to balance engines)
      av0[65,640] = [w0*v | 1]^T @ p0   (denominator Z in row 64)
      av1[64,640] = VR1^T @ p1
      recip = 1/Z                   (DVE reciprocal, reads PSUM row)
      bcr = broadcast(recip)        (gpsimd)
      YT = av0[0:64]*bcr + av1      (DVE mult + add)
      The av matmuls of head h are interleaved between the ST matmuls of
      head h+1 so the PE never idles (keeps tensor-engine p-state high).
  P4: out = YT.T @ proj_w + proj_b  (lhsT = YT tiles directly), DMA out f16.
"""
import math
from contextlib import ExitStack

import numpy as np

import concourse.bacc as bacc
import concourse.bass as bass
import concourse.mybir as mybir
import concourse.tile as tile
from concourse.bass_utils import run_bass_kernel_spmd

F32 = mybir.dt.float32
F16 = mybir.dt.float16
BF16 = mybir.dt.bfloat16
AF = mybir.ActivationFunctionType
ALU = mybir.AluOpType

NCORES = 8
B, N, D, H, DH = 16, 640, 768, 12, 64
BPC = B // NCORES          # batches per core
TOK = BPC * N              # tokens per core (1280)
SCALE = DH ** -0.5
KT = 5                     # 640/128 token tiles per batch
FT = 6                     # 768/128 dim tiles

# p1 = relu(st)^2 path: walrus rejects dual-PSUM scalar_tensor_tensor, so
# two-pass: relu into f16 SBUF (ACT for RELU_ACT_KT + gpsimd square; DVE
# tensor_scalar_max + DVE f16 square otherwise) to balance the engines.
RELU_ACT_KT = (1, 3)       # kt indices routed via ACT relu
SQ_GP_KT = (0, 1, 3)       # kt indices whose square runs on gpsimd
QH = (0, 320)              # q-half offsets for AV accumulation (1 bank each)
RECIP_MODE = "approx_sbuf"  # "exact" | "approx_psum" | "approx_sbuf"

# aux layout inside the combined f16 aux tile (column offsets)
A_ID = 0            # ident [128, 128]
A_ONESR = 128       # ones row [1, 640] (row 0)
A_QKVB = 768        # qkv_b row [1, 2304] (only v part used)
A_PROJB = 3072      # proj_b row [1, 768]
A_ONES12 = 3840     # ones block [128, 12] (for v ones columns)
A_W = 3852          # total f16 aux cols
VW = H * (DH + 1)   # 780: per-(b,kt) v block: 12 heads x [64 feats | ones]


def build_nc(debug=False):
    nc = bacc.Bacc("TRN2", target_bir_lowering=False, debug=False)

    x_d = nc.dram_tensor("x", [TOK, D], F16, kind="ExternalInput")
    qkvw_d = nc.dram_tensor("qkv_w", [D, 3 * D], F16, kind="ExternalInput")
    projw_d = nc.dram_tensor("proj_w", [D, D], F16, kind="ExternalInput")
    auxr_d = nc.dram_tensor("auxr", [128, A_W], F16, kind="ExternalInput")
    auxf_d = nc.dram_tensor("auxf", [128, 14], F32, kind="ExternalInput")
    out_d = nc.dram_tensor("out", [TOK, D], F16, kind="ExternalOutput")

    with tile.TileContext(nc) as tc, ExitStack() as ctx:
        perm = ctx.enter_context(tc.tile_pool(name="perm", bufs=1))
        AX = perm.tile([128, A_W], F16, tag="auxr")
        AXF = perm.tile([128, 14], F32, tag="auxf")
        nc.sync.dma_start(AX[:], auxr_d[:])
        nc.sync.dma_start(AXF[:], auxf_d[:])
        ident = AX[:, A_ID:A_ID + 128]
        onesr = AX[0:1, A_ONESR:A_ONESR + N]
        qkvb = AX[0:1, A_QKVB:A_QKVB + 3 * D]
        projb = AX[0:1, A_PROJB:A_PROJB + D]
        w0c = AXF[:, 0:1]
        w1c = AXF[:, 1:2]          # w1 * SCALE^2
        qkbc = AXF[:, 2:14]        # q/k bias columns (12 x [128,1])

        qv = ctx.enter_context(tc.tile_pool(name="pqv", bufs=1))
        QK = qv.tile([128, BPC * 12 * N], F16, tag="qk")  # (b,f): f<6 q, f>=6 k
        VR = qv.tile([128, BPC * KT * VW], F16, tag="vr")   # [w0*v | 1]
        VR1 = qv.tile([128, BPC * KT * D], F16, tag="vr1")  # w1*SCALE^2*v

        def qk_col(b, f, c):
            return (b * 12 + f) * N + c

        def v_col(b, kt, c):
            return (b * KT + kt) * VW + c

        def v1_col(b, kt, c):
            return (b * KT + kt) * D + c

        # ---------------- P1: load x, transpose per batch ----------------
        with tc.tile_pool(name="pwq", bufs=1) as pwq, \
             tc.tile_pool(name="pxt", bufs=1) as pxt:
            XT = pxt.tile([128, BPC * FT * N], F16, tag="xt")  # [dim, tok]

            def xt_col(b, f, c):
                return (b * FT + f) * N + c

            with tc.tile_pool(name="pxs", bufs=1) as pxs, \
                 tc.tile_pool(name="ps1", bufs=2, space="PSUM") as ps1:
                XS = pxs.tile([128, BPC * KT * D], F16, tag="xs")
                for b in range(BPC):
                    for t in range(KT):
                        nc.sync.dma_start(
                            XS[:, (b * KT + t) * D:(b * KT + t + 1) * D],
                            x_d[b * N + t * 128: b * N + (t + 1) * 128, :])
                WQ = pwq.tile([128, FT * 3 * D], F16, tag="wq")
                for k in range(FT):
                    nc.sync.dma_start(WQ[:, k * 3 * D:(k + 1) * 3 * D],
                                      qkvw_d[k * 128:(k + 1) * 128, :])
                for b in range(BPC):
                    for ft in range(FT):
                        for g in range(0, KT, 4):
                            gw = min(4, KT - g) * 128
                            tp = ps1.tile([128, 512], F16, tag="tp")
                            for j in range(min(4, KT - g)):
                                t = g + j
                                nc.tensor.transpose(
                                    tp[:, j * 128:(j + 1) * 128],
                                    XS[:, (b * KT + t) * D + ft * 128:
                                          (b * KT + t) * D + (ft + 1) * 128],
                                    ident)
                            nc.vector.tensor_copy(
                                XT[:, xt_col(b, ft, g * 128):
                                      xt_col(b, ft, g * 128 + gw)],
                                tp[:, 0:gw])

            # ---------------- P2: qkv projections ----------------
            with tc.tile_pool(name="ps2a", bufs=2, space="PSUM") as ps2a, \
                 tc.tile_pool(name="ps2b", bufs=2, space="PSUM") as ps2b:
                for b in range(BPC):
                    for f in range(12):
                        fcol = f * 128 if f < 6 else 768 + (f - 6) * 128
                        qp = ps2a.tile([128, N], F32, tag="qp")
                        for off, wd in ((0, 512), (512, 128)):
                            for k in range(FT):
                                nc.tensor.matmul(
                                    qp[:, off:off + wd],
                                    WQ[:, k * 3 * D + fcol:
                                          k * 3 * D + fcol + 128],
                                    XT[:, xt_col(b, k, off):
                                          xt_col(b, k, off + wd)],
                                    start=(k == 0), stop=(k == FT - 1))
                        # eviction with bias fold (per-partition bias col)
                        nc.scalar.activation(
                            QK[:, qk_col(b, f, 0):qk_col(b, f, N)], qp[:],
                            AF.Identity, bias=qkbc[:, f:f + 1], scale=1.0)

                    for t in range(KT):
                        vp = ps2b.tile([128, D], F32, tag="vp")
                        for off, wd in ((0, 512), (512, 256)):
                            for k in range(FT):
                                nc.tensor.matmul(
                                    vp[:, off:off + wd],
                                    XT[:, xt_col(b, k, t * 128):
                                          xt_col(b, k, (t + 1) * 128)],
                                    WQ[:, k * 3 * D + 1536 + off:
                                          k * 3 * D + 1536 + off + wd],
                                    start=(k == 0), stop=False)
                            nc.tensor.matmul(
                                vp[:, off:off + wd],
                                onesr[0:1, 0:128],
                                qkvb[0:1, 1536 + off:1536 + off + wd],
                                start=False, stop=True)
                        vdst = VR[:, v_col(b, t, 0):v_col(b, t, VW)] \
                            .rearrange("p (h c) -> p h c", h=H)[:, :, 0:DH]
                        vsrc = vp[:].rearrange("p (h c) -> p h c", h=H)
                        nc.scalar.activation(vdst, vsrc,
                                             AF.Copy, bias=0.0, scale=w0c)
                        vones = VR[:, v_col(b, t, 0):v_col(b, t, VW)] \
                            .rearrange("p (h c) -> p h c", h=H)[:, :, DH:DH + 1]
                        nc.scalar.activation(
                            vones,
                            AX[:, A_ONES12:A_ONES12 + H]
                            .rearrange("p (h c) -> p h c", c=1),
                            AF.Copy, bias=0.0, scale=1.0)
                        nc.scalar.activation(
                            VR1[:, v1_col(b, t, 0):v1_col(b, t, D)], vp[:],
                            AF.Copy, bias=0.0, scale=w1c)

        # ---------------- P3: attention per (b, h) ----------------
        with tc.tile_pool(name="pyt", bufs=1) as pyt:
            YT = pyt.tile([128, BPC * 6 * N], F16, tag="yt")

            def yt_col(b, pi, c):
                return (b * 6 + pi) * N + c

            heads = [(b, h) for b in range(BPC) for h in range(H)]

            with tc.tile_pool(name="pp0", bufs=15) as pp0, \
                 tc.tile_pool(name="ppr", bufs=8) as ppr, \
                 tc.tile_pool(name="pp1", bufs=15) as pp1, \
                 tc.tile_pool(name="psm", bufs=12) as psm, \
                 tc.tile_pool(name="ps3st", bufs=2, space="PSUM") as ps3st, \
                 tc.tile_pool(name="ps3a", bufs=2, space="PSUM") as ps3a, \
                 tc.tile_pool(name="ps3b", bufs=2, space="PSUM") as ps3b:

                def emit_scores(b, h):
                    """ST matmuls + p0/p1 production for head (b,h)."""
                    p0s, p1s = [], []
                    pi, po = h // 2, 64 * (h % 2)
                    for kt in range(KT):
                        st = ps3st.tile([128, N], F32, tag="st")
                        for off, wd in ((0, 512), (512, 128)):
                            nc.tensor.matmul(
                                st[:, off:off + wd],
                                QK[po:po + 64,
                                   qk_col(b, 6 + pi, kt * 128):
                                   qk_col(b, 6 + pi, (kt + 1) * 128)],
                                QK[po:po + 64,
                                   qk_col(b, pi, off):
                                   qk_col(b, pi, off + wd)],
                                start=True, stop=True)
                        p0 = pp0.tile([128, N], F16, tag="p0")
                        nc.scalar.activation(p0[:], st[:], AF.Exp,
                                             bias=0.0, scale=SCALE)
                        p1 = pp1.tile([128, N], F16, tag="p1")
                        r = ppr.tile([128, N], F16, tag="r")
                        if kt in RELU_ACT_KT:
                            nc.scalar.activation(r[:], st[:], AF.Relu,
                                                 bias=0.0, scale=1.0)
                        else:
                            nc.vector.tensor_scalar_max(r[:], st[:], 0.0)
                        if kt in SQ_GP_KT:
                            nc.gpsimd.tensor_tensor(p1[:], r[:], r[:],
                                                    ALU.mult)
                        else:
                            nc.vector.tensor_tensor(p1[:], r[:], r[:],
                                                    ALU.mult)
                        p0s.append(p0)
                        p1s.append(p1)
                    return p0s, p1s

                def emit_av_half(b, h, qo, av0, av1, p0s, p1s):
                    """AV accumulation for one q-half [qo, qo+320)."""
                    sl = slice(qo, qo + 320)
                    for kt in range(KT):
                        nc.tensor.matmul(
                            av0[0:65, :],
                            VR[:, v_col(b, kt, h * (DH + 1)):
                                  v_col(b, kt, h * (DH + 1) + DH + 1)],
                            p0s[kt][:, sl],
                            start=(kt == 0), stop=(kt == KT - 1))
                        nc.tensor.matmul(
                            av1[0:64, :],
                            VR1[:, v1_col(b, kt, h * DH):
                                   v1_col(b, kt, h * DH + DH)],
                            p1s[kt][:, sl],
                            start=(kt == 0), stop=(kt == KT - 1))

                # Three-stage software pipeline over heads:
                #   iter i: blend_post(h[i-3]); { st(h[i],kt) + p0/p1(h[i],kt)
                #           + av-step(h[i-2],kt) interleaved per kt };
                #           blend_pre(h[i-2])
                # AV matmuls run 2 heads behind the scores so every p0/p1
                # they need is long finished (no PE head-of-line stalls),
                # and the blend chain (zrow->recip->bcast->mult/add) spans
                # a full iteration so no engine waits on another in-flight.
                def emit_blend_pre(state, half):
                    (pb, ph), pav = state["head"], state["av"]
                    av0, av1 = pav[half]
                    recip = psm.tile([1, 320], F32, tag="recip")
                    if RECIP_MODE == "approx_psum":
                        nc.vector.reciprocal_approx_fast(recip[:],
                                                         av0[64:65, :])
                    elif RECIP_MODE == "approx_sbuf":
                        zrow = psm.tile([1, 320], F32, tag="zrow")
                        nc.scalar.activation(zrow[:], av0[64:65, :],
                                             AF.Identity, bias=0.0, scale=1.0)
                        nc.vector.reciprocal_approx_fast(recip[:], zrow[:])
                    else:
                        nc.vector.reciprocal(recip[:], av0[64:65, :])
                    bcr = psm.tile([64, 320], F32, tag="bcr")
                    nc.gpsimd.partition_broadcast(bcr[:], recip[:])
                    state["bcr"][half] = bcr

                def emit_blend_post(state, half):
                    (pb, ph), pav = state["head"], state["av"]
                    av0, av1 = pav[half]
                    bcr = state["bcr"][half]
                    ppi, ppo = ph // 2, 64 * (ph % 2)
                    qo = QH[half]
                    t0 = psm.tile([64, 320], F16, tag="t0")
                    nc.vector.tensor_tensor(t0[:], av0[0:64, :], bcr[:],
                                            ALU.mult)
                    nc.vector.tensor_tensor(
                        YT[ppo:ppo + 64,
                           yt_col(pb, ppi, qo):yt_col(pb, ppi, qo + 320)],
                        t0[:], av1[0:64, :], ALU.add)

                def emit_av_step(state, kt):
                    (ab, ah) = state["head"]
                    p0s, p1s = state["p0s"], state["p1s"]
                    if kt == 0:
                        state["av"] = [
                            (ps3a.tile([65, 320], F32, tag="av0",
                                       name=f"av0_{ab}_{ah}_{half}"),
                             ps3b.tile([64, 320], F32, tag="av1",
                                       name=f"av1_{ab}_{ah}_{half}"))
                            for half in (0, 1)]
                    for half in (0, 1):
                        av0, av1 = state["av"][half]
                        sl = slice(QH[half], QH[half] + 320)
                        nc.tensor.matmul(
                            av0[0:65, :],
                            VR[:, v_col(ab, kt, ah * (DH + 1)):
                                  v_col(ab, kt, ah * (DH + 1) + DH + 1)],
                            p0s[kt][:, sl],
                            start=(kt == 0), stop=(kt == KT - 1))
                        nc.tensor.matmul(
                            av1[0:64, :],
                            VR1[:, v1_col(ab, kt, ah * DH):
                                   v1_col(ab, kt, ah * DH + DH)],
                            p1s[kt][:, sl],
                            start=(kt == 0), stop=(kt == KT - 1))

                def emit_head(cur, avst):
                    """Scores for head cur, interleaving AV steps of avst."""
                    b, h = cur
                    pi, po = h // 2, 64 * (h % 2)
                    p0s, p1s = [], []
                    for kt in range(KT):
                        st = ps3st.tile([128, N], F32, tag="st")
                        for off, wd in ((0, 512), (512, 128)):
                            nc.tensor.matmul(
                                st[:, off:off + wd],
                                QK[po:po + 64,
                                   qk_col(b, 6 + pi, kt * 128):
                                   qk_col(b, 6 + pi, (kt + 1) * 128)],
                                QK[po:po + 64,
                                   qk_col(b, pi, off):
                                   qk_col(b, pi, off + wd)],
                                start=True, stop=True)
                        if avst is not None:
                            emit_av_step(avst, kt)
                        p0 = pp0.tile([128, N], F16, tag="p0")
                        nc.scalar.activation(p0[:], st[:], AF.Exp,
                                             bias=0.0, scale=SCALE)
                        p1 = pp1.tile([128, N], F16, tag="p1")
                        r = ppr.tile([128, N], F16, tag="r")
                        if kt in RELU_ACT_KT:
                            nc.scalar.activation(r[:], st[:], AF.Relu,
                                                 bias=0.0, scale=1.0)
                        else:
                            nc.vector.tensor_scalar_max(r[:], st[:], 0.0)
                        if kt in SQ_GP_KT:
                            nc.gpsimd.tensor_tensor(p1[:], r[:], r[:],
                                                    ALU.mult)
                        else:
                            nc.vector.tensor_tensor(p1[:], r[:], r[:],
                                                    ALU.mult)
                        p0s.append(p0)
                        p1s.append(p1)
                    return p0s, p1s

                # pipeline registers: sc1/sc2 = heads whose scores are done,
                # bl = head whose avs are done (blend_pre emitted, post due)
                sc1 = sc2 = bl = None
                for cur in heads + [None, None, None]:
                    if bl is not None:
                        for half in (0, 1):
                            emit_blend_post(bl, half)
                        bl = None
                    if cur is not None:
                        p0s, p1s = emit_head(cur, sc2)
                    elif sc2 is not None:
                        for kt in range(KT):
                            emit_av_step(sc2, kt)
                    if sc2 is not None:
                        for half in (0, 1):
                            emit_blend_pre(sc2, half)
                        bl = sc2
                    sc2 = sc1
                    sc1 = ({"head": cur, "p0s": p0s, "p1s": p1s,
                            "av": None, "bcr": [None, None]}
                           if cur is not None else None)

            # ---------------- P4: proj ----------------
            with tc.tile_pool(name="pw2", bufs=1) as pw2, \
                 tc.tile_pool(name="ps4", bufs=4, space="PSUM") as ps4:
                PW = pw2.tile([128, FT * D], F16, tag="pw")
                OUTS = pw2.tile([128, BPC * KT * D], F16, tag="outs")
                for k in range(FT):
                    nc.sync.dma_start(PW[:, k * D:(k + 1) * D],
                                      projw_d[k * 128:(k + 1) * 128, :])
                for b in range(BPC):
                    for t in range(KT):
                        op = ps4.tile([128, D], F32, tag="op")
                        for off, wd in ((0, 512), (512, 256)):
                            for f in range(FT):
                                nc.tensor.matmul(
                                    op[:, off:off + wd],
                                    YT[:, (b * 6 + f) * N + t * 128:
                                          (b * 6 + f) * N + (t + 1) * 128],
                                    PW[:, f * D + off:
                                          f * D + off + wd],
                                    start=(f == 0), stop=False)
                            nc.tensor.matmul(
                                op[:, off:off + wd],
                                onesr[0:1, 0:128],
                                projb[0:1, off:off + wd],
                                start=False, stop=True)
                        g = b * KT + t
                        if g % 2 == 0:
                            nc.vector.tensor_copy(
                                OUTS[:, g * D:(g + 1) * D], op[:])
                        else:
                            nc.scalar.activation(
                                OUTS[:, g * D:(g + 1) * D], op[:],
                                AF.Copy, bias=0.0, scale=1.0)
                        nc.sync.dma_start(out_d[g * 128:(g + 1) * 128, :],
                                          OUTS[:, g * D:(g + 1) * D])

    nc.compile()
    return nc


_NC_CACHE = None


def _get_nc():
    global _NC_CACHE
    if _NC_CACHE is None:
        _NC_CACHE = build_nc()
    return _NC_CACHE


def kernel(x, qkv_w, qkv_b, proj_w, proj_b, w, t_h=8, t_w=8, s_h=24, s_w=24):
    x = np.asarray(x, dtype=np.float32)
    qkv_w = np.asarray(qkv_w, dtype=np.float32)
    qkv_b = np.asarray(qkv_b, dtype=np.float32)
    proj_w = np.asarray(proj_w, dtype=np.float32)
    proj_b = np.asarray(proj_b, dtype=np.float32)
    w = np.asarray(w, dtype=np.float32)

    we = np.exp(w - w.max())
    ws = we / we.sum()
    w0, w1 = float(ws[0]), float(ws[1])

    auxr = np.zeros((128, A_W), np.float32)
    auxr[:, A_ID:A_ID + 128] = np.eye(128, dtype=np.float32)
    auxr[0, A_ONESR:A_ONESR + N] = 1.0
    auxr[0, A_QKVB:A_QKVB + 3 * D] = qkv_b
    auxr[0, A_PROJB:A_PROJB + D] = proj_b
    auxr[:, A_ONES12:A_ONES12 + H] = 1.0
    auxf = np.zeros((128, 14), np.float32)
    auxf[:, 0] = w0
    auxf[:, 1] = w1 * SCALE * SCALE
    for j in range(12):
        fcol = j * 128 if j < 6 else 768 + (j - 6) * 128
        auxf[:, 2 + j] = qkv_b[fcol:fcol + 128]

    common = {"qkv_w": qkv_w.astype(np.float16),
              "proj_w": proj_w.astype(np.float16),
              "auxr": auxr.astype(np.float16), "auxf": auxf}
    in_maps = []
    for c in range(NCORES):
        m = dict(common)
        m["x"] = np.ascontiguousarray(
            x[c * BPC:(c + 1) * BPC].reshape(TOK, D)).astype(np.float16)
        in_maps.append(m)

    nc = _get_nc()
    res = run_bass_kernel_spmd(nc, in_maps, core_ids=list(range(NCORES)))
    out = np.concatenate(
        [r["out"].reshape(BPC, N, D) for r in res.results], axis=0)
    return out.astype(np.float32)


# revision 26
# speedup vs baseline: 1.9812x; 1.9812x over previous
"""AttentionSSA Trainium2 Bass kernel.

Computation (per batch b):
  qkv = x @ qkv_w + qkv_b ; split into per-head q,k,v
  S = (q @ k^T) * scale
  attn = softmax(w)[0] * softmax(S) + softmax(w)[1] * relu(S)^2
  out = (attn @ v) reassembled, @ proj_w + proj_b

Sharding: data-parallel over batch B=16 across 8 NeuronCores (2 batches/core).
Each core computes its slice fully independently (no collectives).

Per-core dataflow:
  P1: x_b [640,768] --PE transpose--> xT_b [768,640]
  P2: qT,kT = (qkv_w.T @ x.T) per feature tile (T-orientation, [feat, tok]);
      bias folded into the PSUM->SBUF eviction (ACT Identity with per-
      partition bias column). v in natural orientation [tok, feat];
      evicted twice: VR0 = [w0*v | 1] (per-head strided) and
      VR1 = (w1*SCALE^2)*v (contiguous).
  P3: per (b, h): ST[ktok, qtok] = k q^T;
      p0 = exp(SCALE*ST)            (ACT, fused scale)
      p1 = relu(ST)^2 = (ST max 0)*ST  (DVE fused scalar_tensor_tensor;
                                        one kt per head via ACT relu +
                                        gpsimd square to balance engines)
      av0[65,640] = [w0*v | 1]^T @ p0   (denominator Z in row 64)
      av1[64,640] = VR1^T @ p1
      recip = 1/Z                   (DVE reciprocal, reads PSUM row)
      bcr = broadcast(recip)        (gpsimd)
      YT = av0[0:64]*bcr + av1      (DVE mult + add)
      The av matmuls of head h are interleaved between the ST matmuls of
      head h+1 so the PE never idles (keeps tensor-engine p-state high).
  P4: out = YT.T @ proj_w + proj_b  (lhsT = YT tiles directly), DMA out f16.
"""
import math
from contextlib import ExitStack

import numpy as np

import concourse.bacc as bacc
import concourse.bass as bass
import concourse.mybir as mybir
import concourse.tile as tile
from concourse.bass_utils import run_bass_kernel_spmd

F32 = mybir.dt.float32
F16 = mybir.dt.float16
BF16 = mybir.dt.bfloat16
AF = mybir.ActivationFunctionType
ALU = mybir.AluOpType

NCORES = 8
B, N, D, H, DH = 16, 640, 768, 12, 64
BPC = B // NCORES          # batches per core
TOK = BPC * N              # tokens per core (1280)
SCALE = DH ** -0.5
KT = 5                     # 640/128 token tiles per batch
FT = 6                     # 768/128 dim tiles

# p1 = relu(st)^2 path: walrus rejects dual-PSUM scalar_tensor_tensor, so
# two-pass: relu into f16 SBUF (ACT for RELU_ACT_KT + gpsimd square; DVE
# tensor_scalar_max + DVE f16 square otherwise) to balance the engines.
RELU_ACT_KT = (1, 3)       # kt indices routed via ACT relu
SQ_GP_KT = (0, 1, 3)       # kt indices whose square runs on gpsimd
QH = (0, 320)              # q-half offsets for AV accumulation (1 bank each)
RECIP_MODE = "approx_sbuf"  # "exact" | "approx_psum" | "approx_sbuf"
BCAST_MODE = "dma"         # "dma" (DRAM round trip) | "gp" (partition_broadcast)

# aux layout inside the combined f16 aux tile (column offsets)
A_ID = 0            # ident [128, 128]
A_ONESR = 128       # ones row [1, 640] (row 0)
A_QKVB = 768        # qkv_b row [1, 2304] (only v part used)
A_PROJB = 3072      # proj_b row [1, 768]
A_ONES12 = 3840     # ones block [128, 12] (for v ones columns)
A_W = 3852          # total f16 aux cols
VW = H * (DH + 1)   # 780: per-(b,kt) v block: 12 heads x [64 feats | ones]


def build_nc(debug=False):
    nc = bacc.Bacc("TRN2", target_bir_lowering=False, debug=False)

    x_d = nc.dram_tensor("x", [TOK, D], F16, kind="ExternalInput")
    qkvw_d = nc.dram_tensor("qkv_w", [D, 3 * D], F16, kind="ExternalInput")
    projw_d = nc.dram_tensor("proj_w", [D, D], F16, kind="ExternalInput")
    auxr_d = nc.dram_tensor("auxr", [128, A_W], F16, kind="ExternalInput")
    auxf_d = nc.dram_tensor("auxf", [128, 14], F32, kind="ExternalInput")
    out_d = nc.dram_tensor("out", [TOK, D], F16, kind="ExternalOutput")
    zd_d = nc.dram_tensor("zd", [BPC * H, N], F32, kind="Internal")

    with tile.TileContext(nc) as tc, ExitStack() as ctx:
        perm = ctx.enter_context(tc.tile_pool(name="perm", bufs=1))
        AX = perm.tile([128, A_W], F16, tag="auxr")
        AXF = perm.tile([128, 14], F32, tag="auxf")
        nc.sync.dma_start(AX[:], auxr_d[:])
        nc.sync.dma_start(AXF[:], auxf_d[:])
        ident = AX[:, A_ID:A_ID + 128]
        onesr = AX[0:1, A_ONESR:A_ONESR + N]
        qkvb = AX[0:1, A_QKVB:A_QKVB + 3 * D]
        projb = AX[0:1, A_PROJB:A_PROJB + D]
        w0c = AXF[:, 0:1]
        w1c = AXF[:, 1:2]          # w1 * SCALE^2
        qkbc = AXF[:, 2:14]        # q/k bias columns (12 x [128,1])

        qv = ctx.enter_context(tc.tile_pool(name="pqv", bufs=1))
        QK = qv.tile([128, BPC * 12 * N], F16, tag="qk")  # (b,f): f<6 q, f>=6 k
        VR = qv.tile([128, BPC * KT * VW], F16, tag="vr")   # [w0*v | 1]
        VR1 = qv.tile([128, BPC * KT * D], F16, tag="vr1")  # w1*SCALE^2*v

        def qk_col(b, f, c):
            return (b * 12 + f) * N + c

        def v_col(b, kt, c):
            return (b * KT + kt) * VW + c

        def v1_col(b, kt, c):
            return (b * KT + kt) * D + c

        # ---------------- P1: load x, transpose per batch ----------------
        with tc.tile_pool(name="pwq", bufs=1) as pwq, \
             tc.tile_pool(name="pxt", bufs=1) as pxt:
            XT = pxt.tile([128, BPC * FT * N], F16, tag="xt")  # [dim, tok]

            def xt_col(b, f, c):
                return (b * FT + f) * N + c

            with tc.tile_pool(name="pxs", bufs=1) as pxs, \
                 tc.tile_pool(name="ps1", bufs=2, space="PSUM") as ps1:
                XS = pxs.tile([128, BPC * KT * D], F16, tag="xs")
                for b in range(BPC):
                    for t in range(KT):
                        nc.sync.dma_start(
                            XS[:, (b * KT + t) * D:(b * KT + t + 1) * D],
                            x_d[b * N + t * 128: b * N + (t + 1) * 128, :])
                WQ = pwq.tile([128, FT * 3 * D], F16, tag="wq")
                for k in range(FT):
                    nc.sync.dma_start(WQ[:, k * 3 * D:(k + 1) * 3 * D],
                                      qkvw_d[k * 128:(k + 1) * 128, :])
                for b in range(BPC):
                    for ft in range(FT):
                        for g in range(0, KT, 4):
                            gw = min(4, KT - g) * 128
                            tp = ps1.tile([128, 512], F16, tag="tp")
                            for j in range(min(4, KT - g)):
                                t = g + j
                                nc.tensor.transpose(
                                    tp[:, j * 128:(j + 1) * 128],
                                    XS[:, (b * KT + t) * D + ft * 128:
                                          (b * KT + t) * D + (ft + 1) * 128],
                                    ident)
                            nc.vector.tensor_copy(
                                XT[:, xt_col(b, ft, g * 128):
                                      xt_col(b, ft, g * 128 + gw)],
                                tp[:, 0:gw])

            # ---------------- P2: qkv projections ----------------
            with tc.tile_pool(name="ps2a", bufs=2, space="PSUM") as ps2a, \
                 tc.tile_pool(name="ps2b", bufs=2, space="PSUM") as ps2b:
                for b in range(BPC):
                    for f in range(12):
                        fcol = f * 128 if f < 6 else 768 + (f - 6) * 128
                        qp = ps2a.tile([128, N], F32, tag="qp")
                        for off, wd in ((0, 512), (512, 128)):
                            for k in range(FT):
                                nc.tensor.matmul(
                                    qp[:, off:off + wd],
                                    WQ[:, k * 3 * D + fcol:
                                          k * 3 * D + fcol + 128],
                                    XT[:, xt_col(b, k, off):
                                          xt_col(b, k, off + wd)],
                                    start=(k == 0), stop=(k == FT - 1))
                        # eviction with bias fold (per-partition bias col)
                        nc.scalar.activation(
                            QK[:, qk_col(b, f, 0):qk_col(b, f, N)], qp[:],
                            AF.Identity, bias=qkbc[:, f:f + 1], scale=1.0)

                    for t in range(KT):
                        vp = ps2b.tile([128, D], F32, tag="vp")
                        for off, wd in ((0, 512), (512, 256)):
                            for k in range(FT):
                                nc.tensor.matmul(
                                    vp[:, off:off + wd],
                                    XT[:, xt_col(b, k, t * 128):
                                          xt_col(b, k, (t + 1) * 128)],
                                    WQ[:, k * 3 * D + 1536 + off:
                                          k * 3 * D + 1536 + off + wd],
                                    start=(k == 0), stop=False)
                            nc.tensor.matmul(
                                vp[:, off:off + wd],
                                onesr[0:1, 0:128],
                                qkvb[0:1, 1536 + off:1536 + off + wd],
                                start=False, stop=True)
                        vdst = VR[:, v_col(b, t, 0):v_col(b, t, VW)] \
                            .rearrange("p (h c) -> p h c", h=H)[:, :, 0:DH]
                        vsrc = vp[:].rearrange("p (h c) -> p h c", h=H)
                        nc.scalar.activation(vdst, vsrc,
                                             AF.Copy, bias=0.0, scale=w0c)
                        vones = VR[:, v_col(b, t, 0):v_col(b, t, VW)] \
                            .rearrange("p (h c) -> p h c", h=H)[:, :, DH:DH + 1]
                        nc.scalar.activation(
                            vones,
                            AX[:, A_ONES12:A_ONES12 + H]
                            .rearrange("p (h c) -> p h c", c=1),
                            AF.Copy, bias=0.0, scale=1.0)
                        nc.scalar.activation(
                            VR1[:, v1_col(b, t, 0):v1_col(b, t, D)], vp[:],
                            AF.Copy, bias=0.0, scale=w1c)

        # ---------------- P3: attention per (b, h) ----------------
        with tc.tile_pool(name="pyt", bufs=1) as pyt:
            YT = pyt.tile([128, BPC * 6 * N], F16, tag="yt")

            def yt_col(b, pi, c):
                return (b * 6 + pi) * N + c

            heads = [(b, h) for b in range(BPC) for h in range(H)]

            with tc.tile_pool(name="pp0", bufs=15) as pp0, \
                 tc.tile_pool(name="ppr", bufs=8) as ppr, \
                 tc.tile_pool(name="pp1", bufs=15) as pp1, \
                 tc.tile_pool(name="psm", bufs=5) as psm, \
                 tc.tile_pool(name="pzd", bufs=5, space="DRAM") as pzd, \
                 tc.tile_pool(name="ps3st", bufs=2, space="PSUM") as ps3st, \
                 tc.tile_pool(name="ps3a", bufs=1, space="PSUM") as ps3a, \
                 tc.tile_pool(name="ps3b", bufs=1, space="PSUM") as ps3b:

                def emit_scores(b, h):
                    """ST matmuls + p0/p1 production for head (b,h)."""
                    p0s, p1s = [], []
                    pi, po = h // 2, 64 * (h % 2)
                    for kt in range(KT):
                        st = ps3st.tile([128, N], F32, tag="st")
                        for off, wd in ((0, 512), (512, 128)):
                            nc.tensor.matmul(
                                st[:, off:off + wd],
                                QK[po:po + 64,
                                   qk_col(b, 6 + pi, kt * 128):
                                   qk_col(b, 6 + pi, (kt + 1) * 128)],
                                QK[po:po + 64,
                                   qk_col(b, pi, off):
                                   qk_col(b, pi, off + wd)],
                                start=True, stop=True)
                        p0 = pp0.tile([128, N], F16, tag="p0")
                        nc.scalar.activation(p0[:], st[:], AF.Exp,
                                             bias=0.0, scale=SCALE)
                        p1 = pp1.tile([128, N], F16, tag="p1")
                        r = ppr.tile([128, N], F16, tag="r")
                        if kt in RELU_ACT_KT:
                            nc.scalar.activation(r[:], st[:], AF.Relu,
                                                 bias=0.0, scale=1.0)
                        else:
                            nc.vector.tensor_scalar_max(r[:], st[:], 0.0)
                        if kt in SQ_GP_KT:
                            nc.gpsimd.tensor_tensor(p1[:], r[:], r[:],
                                                    ALU.mult)
                        else:
                            nc.vector.tensor_tensor(p1[:], r[:], r[:],
                                                    ALU.mult)
                        p0s.append(p0)
                        p1s.append(p1)
                    return p0s, p1s

                def emit_av_half(b, h, qo, av0, av1, p0s, p1s):
                    """AV accumulation for one q-half [qo, qo+320)."""
                    sl = slice(qo, qo + 320)
                    for kt in range(KT):
                        nc.tensor.matmul(
                            av0[0:65, :],
                            VR[:, v_col(b, kt, h * (DH + 1)):
                                  v_col(b, kt, h * (DH + 1) + DH + 1)],
                            p0s[kt][:, sl],
                            start=(kt == 0), stop=(kt == KT - 1))
                        nc.tensor.matmul(
                            av1[0:64, :],
                            VR1[:, v1_col(b, kt, h * DH):
                                   v1_col(b, kt, h * DH + DH)],
                            p1s[kt][:, sl],
                            start=(kt == 0), stop=(kt == KT - 1))

                # Four-stage software pipeline over heads, per iter i
                # (h = heads[i]):
                #   PE:  AV block for h-2 (all p0/p1 long ready), then STs(h)
                #   ACT: evict av0(h-2)->SBUF f32; exps(h) + relus(h)
                #   DVE: mult/add(h-3) [BCZ arrived via DMA an iter ago];
                #        evict av1(h-2)->SBUF f16; recip(h-2); max/sq(h)
                #   GP:  squares(h) only (mono-op, no library thrash)
                #   DMA: recip row -> DRAM -> broadcast back to [64,N] SBUF
                # AV PSUM tiles are freed by the evictions (same iter), so
                # bufs=1 suffices and no engine chain gates the PE.
                def emit_avs(state):
                    (ab, ah) = state["head"]
                    p0s, p1s = state["p0s"], state["p1s"]
                    av0 = ps3a.tile([65, N], F32, tag="av0")
                    av1 = ps3b.tile([64, N], F32, tag="av1")
                    for kt in range(KT):
                        for off, wd in ((0, 512), (512, 128)):
                            sl = slice(off, off + wd)
                            nc.tensor.matmul(
                                av0[0:65, sl],
                                VR[:, v_col(ab, kt, ah * (DH + 1)):
                                      v_col(ab, kt, ah * (DH + 1) + DH + 1)],
                                p0s[kt][:, sl],
                                start=(kt == 0), stop=(kt == KT - 1))
                            nc.tensor.matmul(
                                av1[0:64, sl],
                                VR1[:, v1_col(ab, kt, ah * DH):
                                       v1_col(ab, kt, ah * DH + DH)],
                                p1s[kt][:, sl],
                                start=(kt == 0), stop=(kt == KT - 1))
                    # evictions free the PSUM slots; recip + broadcast are
                    # off the per-head critical loop (DMA round trip).
                    # Z row is evicted to partition 0 of its own tile: the
                    # custom DVE reciprocal op silently corrupts reads from
                    # a nonzero base partition.
                    a0 = psm.tile([64, N], F32, tag="a0")
                    nc.scalar.activation(a0[:], av0[0:64, :], AF.Copy,
                                         bias=0.0, scale=1.0)
                    zr = psm.tile([1, N], F32, tag="zr")
                    nc.scalar.activation(zr[:], av0[64:65, :], AF.Copy,
                                         bias=0.0, scale=1.0)
                    a1 = psm.tile([64, N], F16, tag="a1")
                    nc.vector.tensor_copy(a1[:], av1[0:64, :])
                    rc = psm.tile([1, N], F32, tag="rc")
                    if RECIP_MODE == "exact":
                        nc.vector.reciprocal(rc[:], zr[0:1, :])
                    else:
                        nc.vector.reciprocal_approx_fast(rc[:], zr[0:1, :])
                    bcz = psm.tile([64, N], F32, tag="bcz")
                    if BCAST_MODE == "dma":
                        zd = pzd.tile([1, N], F32, tag="zd")
                        nc.sync.dma_start(zd[:], rc[:])
                        nc.sync.dma_start(bcz[:],
                                          zd[0:1, :].to_broadcast([64, N]))
                    else:
                        nc.gpsimd.partition_broadcast(bcz[:], rc[:])
                    state["a0"], state["a1"], state["bcz"] = a0, a1, bcz

                def emit_blend_post(state):
                    (pb, ph) = state["head"]
                    a0, a1, bcz = state["a0"], state["a1"], state["bcz"]
                    ppi, ppo = ph // 2, 64 * (ph % 2)
                    t0 = psm.tile([64, N], F16, tag="t0")
                    nc.vector.tensor_tensor(t0[:], a0[0:64, :], bcz[:],
                                            ALU.mult)
                    nc.vector.tensor_tensor(
                        YT[ppo:ppo + 64, yt_col(pb, ppi, 0):yt_col(pb, ppi, N)],
                        t0[:], a1[:], ALU.add)

                def emit_head(cur):
                    b, h = cur
                    pi, po = h // 2, 64 * (h % 2)
                    p0s, p1s = [], []
                    for kt in range(KT):
                        st = ps3st.tile([128, N], F32, tag="st")
                        for off, wd in ((0, 512), (512, 128)):
                            nc.tensor.matmul(
                                st[:, off:off + wd],
                                QK[po:po + 64,
                                   qk_col(b, 6 + pi, kt * 128):
                                   qk_col(b, 6 + pi, (kt + 1) * 128)],
                                QK[po:po + 64,
                                   qk_col(b, pi, off):
                                   qk_col(b, pi, off + wd)],
                                start=True, stop=True)
                        p0 = pp0.tile([128, N], F16, tag="p0")
                        nc.scalar.activation(p0[:], st[:], AF.Exp,
                                             bias=0.0, scale=SCALE)
                        p1 = pp1.tile([128, N], F16, tag="p1")
                        r = ppr.tile([128, N], F16, tag="r")
                        if kt in RELU_ACT_KT:
                            nc.scalar.activation(r[:], st[:], AF.Relu,
                                                 bias=0.0, scale=1.0)
                        else:
                            nc.vector.tensor_scalar_max(r[:], st[:], 0.0)
                        if kt in SQ_GP_KT:
                            nc.gpsimd.tensor_tensor(p1[:], r[:], r[:],
                                                    ALU.mult)
                        else:
                            nc.vector.tensor_tensor(p1[:], r[:], r[:],
                                                    ALU.mult)
                        p0s.append(p0)
                        p1s.append(p1)
                    return p0s, p1s

                sc1 = sc2 = bl = None
                for cur in heads + [None, None, None]:
                    if bl is not None:
                        emit_blend_post(bl)
                        bl = None
                    if sc2 is not None:
                        emit_avs(sc2)
                        bl = sc2
                    if cur is not None:
                        p0s, p1s = emit_head(cur)
                    sc2 = sc1
                    sc1 = ({"head": cur, "p0s": p0s, "p1s": p1s}
                           if cur is not None else None)

            # ---------------- P4: proj ----------------
            with tc.tile_pool(name="pw2", bufs=1) as pw2, \
                 tc.tile_pool(name="ps4", bufs=4, space="PSUM") as ps4:
                PW = pw2.tile([128, FT * D], F16, tag="pw")
                OUTS = pw2.tile([128, BPC * KT * D], F16, tag="outs")
                for k in range(FT):
                    nc.sync.dma_start(PW[:, k * D:(k + 1) * D],
                                      projw_d[k * 128:(k + 1) * 128, :])
                for b in range(BPC):
                    for t in range(KT):
                        op = ps4.tile([128, D], F32, tag="op")
                        for off, wd in ((0, 512), (512, 256)):
                            for f in range(FT):
                                nc.tensor.matmul(
                                    op[:, off:off + wd],
                                    YT[:, (b * 6 + f) * N + t * 128:
                                          (b * 6 + f) * N + (t + 1) * 128],
                                    PW[:, f * D + off:
                                          f * D + off + wd],
                                    start=(f == 0), stop=False)
                            nc.tensor.matmul(
                                op[:, off:off + wd],
                                onesr[0:1, 0:128],
                                projb[0:1, off:off + wd],
                                start=False, stop=True)
                        g = b * KT + t
                        if g % 2 == 0:
                            nc.vector.tensor_copy(
                                OUTS[:, g * D:(g + 1) * D], op[:])
                        else:
                            nc.scalar.activation(
                                OUTS[:, g * D:(g + 1) * D], op[:],
                                AF.Copy, bias=0.0, scale=1.0)
                        nc.sync.dma_start(out_d[g * 128:(g + 1) * 128, :],
                                          OUTS[:, g * D:(g + 1) * D])

    nc.compile()
    return nc


_NC_CACHE = None


def _get_nc():
    global _NC_CACHE
    if _NC_CACHE is None:
        _NC_CACHE = build_nc()
    return _NC_CACHE


def kernel(x, qkv_w, qkv_b, proj_w, proj_b, w, t_h=8, t_w=8, s_h=24, s_w=24):
    x = np.asarray(x, dtype=np.float32)
    qkv_w = np.asarray(qkv_w, dtype=np.float32)
    qkv_b = np.asarray(qkv_b, dtype=np.float32)
    proj_w = np.asarray(proj_w, dtype=np.float32)
    proj_b = np.asarray(proj_b, dtype=np.float32)
    w = np.asarray(w, dtype=np.float32)

    we = np.exp(w - w.max())
    ws = we / we.sum()
    w0, w1 = float(ws[0]), float(ws[1])

    auxr = np.zeros((128, A_W), np.float32)
    auxr[:, A_ID:A_ID + 128] = np.eye(128, dtype=np.float32)
    auxr[0, A_ONESR:A_ONESR + N] = 1.0
    auxr[0, A_QKVB:A_QKVB + 3 * D] = qkv_b
    auxr[0, A_PROJB:A_PROJB + D] = proj_b
    auxr[:, A_ONES12:A_ONES12 + H] = 1.0
    auxf = np.zeros((128, 14), np.float32)
    auxf[:, 0] = w0
    auxf[:, 1] = w1 * SCALE * SCALE
    for j in range(12):
        fcol = j * 128 if j < 6 else 768 + (j - 6) * 128
        auxf[:, 2 + j] = qkv_b[fcol:fcol + 128]

    common = {"qkv_w": qkv_w.astype(np.float16),
              "proj_w": proj_w.astype(np.float16),
              "auxr": auxr.astype(np.float16), "auxf": auxf}
    in_maps = []
    for c in range(NCORES):
        m = dict(common)
        m["x"] = np.ascontiguousarray(
            x[c * BPC:(c + 1) * BPC].reshape(TOK, D)).astype(np.float16)
        in_maps.append(m)

    nc = _get_nc()
    res = run_bass_kernel_spmd(nc, in_maps, core_ids=list(range(NCORES)))
    out = np.concatenate(
        [r["out"].reshape(BPC, N, D) for r in res.results], axis=0)
    return out.astype(np.float32)
